# revision 6
# baseline (speedup 1.0000x reference)
"""AdjustedNonLocalBlock on 8 TRN2 NeuronCores.

Math (per batch b, N = H*W spatial positions):
    theta = theta_w @ x1 + theta_b          [Ci, N]   (queries)
    phi   = phi_w   @ x0 + phi_b            [Ci, N]   (keys)
    g     = g_w     @ x0 + g_b              [Ci, N]   (values)
    f     = theta^T @ phi                   [N, N]
    P     = softmax(f, axis=-1)
    y     = (P @ g^T)^T                     [Ci, N]
    out   = W_w @ y + W_b + x0              [C, N]

Device-side reduction used here:
    f[q,k] = x1[:,q]^T A x0[:,k] + t2[q] + t3[k] + t4,  A = theta_w^T phi_w
  Per-query constants (t2, t4) are softmax-invariant -> dropped.
    t3[k] = v^T x0[:,k],  v = phi_w^T theta_b   (the phi_b part of t3 is
  constant over k as well -> dropped).
  g's bias: since softmax rows sum to 1, y = P @ g_raw^T + g_b, so g_b is
  folded into the output bias  b_out = W_w @ g_b + W_b.

Sharding: core i handles batch i//2, query half i%2 (2048 queries x 4096
keys). Softmax is over keys (fully local) -> no collectives.

Per-core pipeline (all f32 storage, float32r matmuls):
    U  = A @ X0                              [128, 4096]   (TensorE)
    gaug tiles: [t3 | g_raw^T] per 128-key tile, plus a ones column
    S^T tile (kt,qp) = U[:,kt]^T ... wait -- matmul(lhsT=U_kt, rhs=X1h)
                     -> S[k, q] = f[q, k] - t3[k]          (TensorE)
    E = exp(S + t3)   (ScalarE activation, per-partition bias)
    Y'^T[0:64] += g_tile^T E ; Y'^T[64] += ones^T E (= Z)  (TensorE accum)
    y = Y'[0:64] * (1/Z broadcast)                          (DVE)
    out = W_aug @ [y; ones] + x0_res                        (TensorE + DVE)
"""

import numpy as np

import concourse.bacc as bacc
import concourse.mybir as mybir
import concourse.tile as tile
from concourse.bass_utils import run_bass_kernel_spmd

B, C, CI = 4, 128, 64
H, W = 64, 64
N = H * W              # 4096
NCORES = 8
QH = N // 2            # 2048 queries per core
KT = N // 128          # 32 key tiles of 128
GSTR = 66              # per-key-tile stride in gaug: [t3 | g(64) | ones]

F32 = mybir.dt.float32
F32R = mybir.dt.float32r

_CACHE = {}


def _mm(ap):
    """Matmul-feeding tiles are already float32r."""
    return ap


def _f32(ap):
    """f32 view of an f32r tile (for ACT bias reads)."""
    return ap.bitcast(F32)


def _build():
    if "nc" in _CACHE:
        return _CACHE["nc"]

    nc = bacc.Bacc("TRN2", target_bir_lowering=False, debug=False,
                   num_devices=NCORES)
    x0_ext = nc.declare_dram_parameter("x0", [C, N], F32R, isOutput=False)
    x1_ext = nc.declare_dram_parameter("x1h", [C, QH], F32R, isOutput=False)
    res_ext = nc.declare_dram_parameter("res", [C, QH], F32, isOutput=False)
    at_ext = nc.declare_dram_parameter("a_t", [C, C], F32R, isOutput=False)
    gv_ext = nc.declare_dram_parameter("gv", [C, CI + 2], F32R, isOutput=False)
    wa_ext = nc.declare_dram_parameter("w_aug", [CI + 1, C], F32R,
                                       isOutput=False)
    out_ext = nc.declare_dram_parameter("out", [C, QH], F32, isOutput=True)

    EXP = mybir.ActivationFunctionType.Exp

    with tile.TileContext(nc) as tc:
        with (
            tc.tile_pool(name="const", bufs=1) as constp,
            tc.tile_pool(name="data", bufs=1) as datap,
            tc.tile_pool(name="epool", bufs=3) as epool,
            tc.tile_pool(name="spool", bufs=2, space="PSUM") as spool,
            tc.tile_pool(name="ypool", bufs=2, space="PSUM") as ypool,
            tc.tile_pool(name="smallp", bufs=2, space="PSUM") as smallp,
            tc.tile_pool(name="ysbp", bufs=2) as ysbp,
            tc.tile_pool(name="outp", bufs=3) as outp,
            tc.tile_pool(name="rzp", bufs=2) as rzp,
        ):
            # ---- inputs ----
            x0_sb = datap.tile([C, N], F32R)
            nc.sync.dma_start(x0_sb[:], x0_ext[:])
            x1_sb = datap.tile([C, QH], F32R)
            nc.sync.dma_start(x1_sb[:], x1_ext[:])
            res_sb = datap.tile([C, QH], F32)
            nc.sync.dma_start(res_sb[:], res_ext[:])
            at_sb = constp.tile([C, C], F32R)
            nc.sync.dma_start(at_sb[:], at_ext[:])
            gv_sb = constp.tile([C, CI + 2], F32R)
            nc.sync.dma_start(gv_sb[:], gv_ext[:])
            wa_sb = constp.tile([CI + 1, C], F32R)
            nc.sync.dma_start(wa_sb[:], wa_ext[:])
            ones_sb = constp.tile([1, CI], F32)
            nc.vector.memset(ones_sb[:], 1.0)

            U_sb = datap.tile([C, N], F32R)
            gaug_sb = datap.tile([C, KT * GSTR], F32R)
            nc.vector.memset(_f32(gaug_sb[:]), 1.0)  # presets the ones columns
            yaug_sb = datap.tile([CI + 1, QH], F32R)
            nc.vector.memset(_f32(yaug_sb[CI:CI + 1, :]), 1.0)

            def emit_u_chunk(c):
                # U[:, c*512:(c+1)*512] = A @ X0 chunk
                pu = smallp.tile([C, 512], F32, tag="sm")
                nc.tensor.matmul(pu[:], _mm(at_sb[:]),
                                 _mm(x0_sb[:, c * 512:(c + 1) * 512]),
                                 start=True, stop=True)
                nc.vector.tensor_copy(U_sb[:, c * 512:(c + 1) * 512], pu[:])

            def emit_gaug(kt):
                # [t3 | g_raw^T] for key tile kt
                pg = smallp.tile([C, 512], F32, tag="sm")
                nc.tensor.matmul(pg[:, 0:CI + 2],
                                 _mm(x0_sb[:, kt * 128:(kt + 1) * 128]),
                                 _mm(gv_sb[:]), start=True, stop=True)
                nc.vector.tensor_copy(
                    gaug_sb[:, kt * GSTR:kt * GSTR + CI + 1], pg[:, 0:CI + 1])

            # Prologue pieces needed before the first exp
            emit_u_chunk(0)
            emit_gaug(0)
            emit_gaug(1)

            def emit_mm1(qp, kt):
                s = spool.tile([C, 1024], F32)
                q0 = qp * 1024
                lhs = _mm(U_sb[:, kt * 128:(kt + 1) * 128])
                nc.tensor.matmul(s[:, 0:512], lhs,
                                 _mm(x1_sb[:, q0:q0 + 512]),
                                 start=True, stop=True)
                nc.tensor.matmul(s[:, 512:1024], lhs,
                                 _mm(x1_sb[:, q0 + 512:q0 + 1024]),
                                 start=True, stop=True)
                return s

            for qp in range(2):
                ya = ypool.tile([CI + 1, 512], F32, tag="y")
                yb = ypool.tile([CI + 1, 512], F32, tag="y")
                s_cur = emit_mm1(qp, 0)
                for kt in range(KT):
                    e = epool.tile([C, 1024], F32R)
                    nc.scalar.activation(
                        e[:], s_cur[:], EXP,
                        bias=_f32(gaug_sb[:, kt * GSTR:kt * GSTR + 1]))
                    # just-in-time prologue (only during first qp pass)
                    if qp == 0:
                        if kt % 4 == 0 and kt // 4 + 1 < 8:
                            emit_u_chunk(kt // 4 + 1)
                        if kt + 2 < KT:
                            emit_gaug(kt + 2)
                    if kt + 1 < KT:
                        s_cur = emit_mm1(qp, kt + 1)
                    st, sp = kt == 0, kt == KT - 1
                    glhs = _mm(gaug_sb[:, kt * GSTR + 1:kt * GSTR + GSTR])
                    nc.tensor.matmul(ya[:], glhs, _mm(e[:, 0:512]),
                                     start=st, stop=sp)
                    nc.tensor.matmul(yb[:], glhs, _mm(e[:, 512:1024]),
                                     start=st, stop=sp)

                for hh, Y in ((0, ya), (1, yb)):
                    qc = qp * 1024 + hh * 512  # query offset within core
                    ysb = ysbp.tile([CI + 1, 512], F32)
                    nc.vector.tensor_copy(ysb[:], Y[:])
                    rz = rzp.tile([1, 512], F32)
                    nc.vector.reciprocal(rz[:], ysb[CI:CI + 1, :])
                    bc = smallp.tile([C, 512], F32, tag="sm")
                    nc.tensor.matmul(bc[0:CI, :], _mm(ones_sb[:]), _mm(rz[:]),
                                     start=True, stop=True)
                    nc.vector.tensor_mul(yaug_sb[0:CI, qc:qc + 512],
                                         ysb[0:CI, :], bc[0:CI, :])
                    pr = smallp.tile([C, 512], F32, tag="sm")
                    nc.tensor.matmul(pr[:], _mm(wa_sb[:]),
                                     _mm(yaug_sb[:, qc:qc + 512]),
                                     start=True, stop=True)
                    ot = outp.tile([C, 512], F32)
                    nc.vector.tensor_add(ot[:], pr[:], res_sb[:, qc:qc + 512])
                    nc.sync.dma_start(out_ext[:, qc:qc + 512], ot[:])

    nc.compile()
    _CACHE["nc"] = nc
    return nc


def _prep_in_maps(inputs):
    x0 = np.ascontiguousarray(np.asarray(inputs["x0"], np.float32))
    x1 = np.ascontiguousarray(np.asarray(inputs["x1"], np.float32))
    g_w = np.asarray(inputs["g_w"], np.float32)
    g_b = np.asarray(inputs["g_b"], np.float32)
    theta_w = np.asarray(inputs["theta_w"], np.float32)
    theta_b = np.asarray(inputs["theta_b"], np.float32)
    phi_w = np.asarray(inputs["phi_w"], np.float32)
    W_w = np.asarray(inputs["W_w"], np.float32)
    W_b = np.asarray(inputs["W_b"], np.float32)

    a_t = np.ascontiguousarray(phi_w.T @ theta_w)            # [C, C]
    v = phi_w.T @ theta_b                                    # [C]
    gv = np.ascontiguousarray(np.concatenate(
        [v[:, None], g_w.T, np.zeros((C, 1), np.float32)], axis=1))  # [C, 66]
    b_out = W_w @ g_b + W_b                                  # [C]
    w_aug = np.ascontiguousarray(
        np.concatenate([W_w.T, b_out[None, :]], axis=0))     # [65, C]

    in_maps = []
    for core in range(NCORES):
        b, hh = core // 2, core % 2
        x0f = x0[b].reshape(C, N)
        x1f = x1[b].reshape(C, N)
        in_maps.append({
            "x0": x0f,
            "x1h": np.ascontiguousarray(x1f[:, hh * QH:(hh + 1) * QH]),
            "res": np.ascontiguousarray(x0f[:, hh * QH:(hh + 1) * QH]),
            "a_t": a_t,
            "gv": gv,
            "w_aug": w_aug,
        })
    return in_maps


def _run(inputs, trace=False):
    nc = _build()
    in_maps = _prep_in_maps(inputs)
    res = run_bass_kernel_spmd(nc, in_maps, core_ids=list(range(NCORES)),
                               trace=trace)
    out = np.empty((B, C, N), np.float32)
    for core in range(NCORES):
        b, hh = core // 2, core % 2
        out[b][:, hh * QH:(hh + 1) * QH] = res.results[core]["out"]
    return out.reshape(B, C, H, W), res


def kernel(**inputs) -> np.ndarray:
    out, _ = _run(inputs, trace=False)
    return out
